# revision 4
# baseline (speedup 1.0000x reference)
"""Trainium2 Bass kernel for routed-token transformer block (moe_routing).

v8 = v7 + sorted-prefix compaction:
  Host sorts each sample's tokens by router weight w = x@wp_w + wp_b
  (descending). Since k >= 1 always (clip lower bound), selected tokens
  {w > k} are a subset of {w > 1}, which is a prefix of the sort. The
  device runs the transformer block only on the first CP tokens (CP =
  max per-sample count rounded up to 128, with a safety margin); the
  host passes through x for the rest. Selection itself (k, and w > k)
  is still computed exactly on device, so any k >= 1 is handled.

  Also: fp8 router stream in 16 large DMAs (large per-partition
  descriptors; small DMAs are descriptor-latency-bound on this runtime),
  bf16 matmul path, single x load / out store.
"""

import ml_dtypes
import numpy as np

import concourse.bass as bass
import concourse.bacc as bacc
import concourse.mybir as mybir
import concourse.tile as tile
from concourse.masks import make_identity

F32 = mybir.dt.float32
BF16 = mybir.dt.bfloat16
FP8 = mybir.dt.float8e4
AF = mybir.ActivationFunctionType
OP = mybir.AluOpType

FULL_CFG = dict(S=2048, D=256, H=8, MLP=1024, KH=512, B=8)
NEG_EPS = 1e-30
GELU_C = 0.7978845608028654  # sqrt(2/pi)
SX = 16.0       # fp8 scale on x columns of the router stream
SW = 4096.0     # fp8 scale on kp_w1
G = 32          # router chunks per DMA
W_MARGIN = 1e-3  # host/device weight-computation slack for the prefix cut


def _bcast_ap(handle, p):
    ap = handle.ap()
    return bass.AP(tensor=ap.tensor, offset=ap.offset,
                   ap=[[0, p]] + [list(x) for x in ap.ap[1:]])


def build_bass(cfg, CP, repeat=1):
    S, D, H, MLP_, KH, B = (cfg[k] for k in ("S", "D", "H", "MLP", "KH", "B"))
    P = 128
    HD = D // H
    CT = CP // P                     # compacted token tiles
    DT = D // P
    KSLICE = S * D // B
    KT = KSLICE // P
    HPT = P // HD
    NSQC = max(1, (CP + 1023) // 1024)
    CH = CP // NSQC                  # attention query-chunk width
    # matmul output pieces must lie within one PSUM bank (512 f32, 512-aligned)
    PIECES = [(s, min(512, CH - s)) for s in range(0, CH, 512)]
    NQC = max(1, (CP + 511) // 512)
    QN = CP // NQC                   # qkT chunk width
    MT = MLP_ // P
    QKM = 2 * D // P
    W1COLS = B + KH
    NQT = (H + 2) // 3
    PSW = 1024

    nc = bacc.Bacc()

    dp = lambda name, shape, dt=F32: nc.declare_dram_parameter(name, list(shape), dt, isOutput=False)
    xs_d = dp("x_sel_pm", (P, CT * D))          # gathered prefix tokens, [p, t*D+c]
    w1x_d = dp("w1x", (P, KT * W1COLS), FP8)
    amask_d = dp("amask_sel", (P, CT))
    WCOLS = QKM * P * DT // DT * DT  # placeholder, set below
    WCOLS = DT * QKM * P + DT * D + DT * D + DT * MT * P + MT * D
    wpack_d = dp("wpack", (P, WCOLS), BF16)   # wqk|wv|wo|wm1|wm2 tiles, row-major per tile
    RCOLS = 9 * D + 2 * KH + 2
    rowpack_d = dp("rowpack", (1, RCOLS))     # ln1g|ln1b|ln2g|ln2b|wpw|bv|bo|bm2|? see marshal
    colpack_d = dp("colpack", (P, QKM + MT))  # bqk (4) | bm1 (8) column slices
    E_d = dp("Emats", (H, DT * P), BF16)
    iota_d = dp("iota_row", (4, S // 4))
    out_d = nc.declare_dram_parameter("out", [P, CT * D], F32, isOutput=True)

    rs_in = nc.dram_tensor("rs_in", [B, KH], F32)
    rs_out = nc.dram_tensor("rs_out", [1, KH], F32)

    with tile.TileContext(nc) as tc:
        with (
            tc.tile_pool(name="singles", bufs=1) as SG,
            tc.tile_pool(name="w1chunk", bufs=2) as W1P,
            tc.tile_pool(name="xsb", bufs=1) as XSB,
            tc.tile_pool(name="tmp", bufs=3) as TMP,
            tc.tile_pool(name="hT", bufs=DT) as HTP,
            tc.tile_pool(name="qk", bufs=1) as QKP,
            tc.tile_pool(name="vaug", bufs=CT) as VAP,
            tc.tile_pool(name="expT", bufs=2) as EXP,
            tc.tile_pool(name="attnT", bufs=1) as ATP,
            tc.tile_pool(name="m1", bufs=2) as M1P,
            tc.tile_pool(name="acts", bufs=CT) as APL,
            tc.tile_pool(name="small", bufs=1) as SM,
            tc.tile_pool(name="ps", bufs=4, space="PSUM") as PS,
        ):
          for _rep in range(repeat):
            pst = lambda pp=P: PS.tile([pp, PSW], F32, tag="ps", name="ps")
            pstb = lambda: PS.tile([P, PSW * 2], BF16, tag="ps", name="ps")
            # ---------------- constants ----------------
            ident = SG.tile([P, P], BF16, tag="ident")
            make_identity(nc, ident)
            eps_t = SG.tile([P, 1], F32, tag="eps")
            nc.vector.memset(eps_t, 1e-5)

            def bload(handle, tag):
                t = SG.tile([P, handle.ap().ap[-1][1]], F32, tag=tag, name=tag)
                nc.sync.dma_start(out=t, in_=_bcast_ap(handle, P))
                return t

            rowp = SG.tile([P, RCOLS], F32, tag="rowp", name="rowp")
            nc.sync.dma_start(out=rowp, in_=_bcast_ap(rowpack_d, P))
            roff = [0]
            def rslice(w):
                o = roff[0]; roff[0] += w
                return rowp[:, o:o + w]
            ln1g_b = rslice(D); ln1b_b = rslice(D)
            ln2g_b = rslice(D); ln2b_b = rslice(D)
            wpw_b = rslice(D); bv_b = rslice(D)
            bo_b = rslice(D); bm2_b = rslice(D)
            wpb_b = rslice(D)                   # wp_b replicated across D cols
            kpb1_sb = rslice(KH)[0:1, :]
            kpw2_sb = rslice(KH)[0:1, :]
            kpb2_sb = rslice(1)[0:1, :]
            _pad1 = rslice(1)

            colp = SG.tile([P, QKM + MT], F32, tag="colp", name="colp")
            nc.sync.dma_start(out=colp, in_=colpack_d.ap())
            bqk_sb = [colp[:, mt:mt + 1] for mt in range(QKM)]
            bm1_sb = [colp[:, QKM + mt:QKM + mt + 1] for mt in range(MT)]

            ones_row = SG.tile([1, P], F32, tag="ones_row")
            nc.vector.memset(ones_row, 1.0)
            E_all = SG.tile([H, DT * P], BF16, tag="Eall", name="Eall")
            nc.sync.dma_start(out=E_all, in_=E_d.ap())
            E_sb = [E_all[:, ht * P:(ht + 1) * P] for ht in range(DT)]
            amask_sb = SG.tile([P, CT], F32, tag="amask")
            nc.sync.dma_start(out=amask_sb, in_=amask_d.ap())
            iota_sb = SG.tile([4, S // 4], F32, tag="iota")
            nc.sync.dma_start(out=iota_sb, in_=iota_d.ap())
            ones4 = SG.tile([4, 1], F32, tag="ones4")
            nc.vector.memset(ones4, 1.0)

            # ---------------- x_sel (one DMA) + block weights (bf16) -------
            x_sb = XSB.tile([P, CT * D], F32, tag="xsb", name="xsb")
            nc.sync.dma_start(out=x_sb, in_=xs_d.ap())
            xt = lambda t: x_sb[:, t * D:(t + 1) * D]

            wpk = SG.tile([P, WCOLS], BF16, tag="wpk", name="wpk")
            nc.sync.dma_start(out=wpk, in_=wpack_d.ap())
            woff = [0]
            def wslice(w):
                o = woff[0]; woff[0] += w
                return wpk[:, o:o + w]
            wqk_sb = {(kt, mt): wslice(P) for kt in range(DT) for mt in range(QKM)}
            wv_sb = {kt: wslice(D) for kt in range(DT)}
            wo_sb = {kt: wslice(D) for kt in range(DT)}
            wm1_sb = {(kt, mt): wslice(P) for kt in range(DT) for mt in range(MT)}
            wm2_sb = {mt: wslice(D) for mt in range(MT)}

            # ---------------- router stream (fp8, G chunks per DMA) --------
            ps_router = pst(B)
            for kg in range(KT // G):
                chunk = W1P.tile([P, G * W1COLS], FP8, tag="w1c", name="w1c")
                nc.sync.dma_start(out=chunk, in_=w1x_d.ap()[:, kg * G * W1COLS:(kg + 1) * G * W1COLS])
                for g in range(G):
                    kt = kg * G + g
                    nc.tensor.matmul(ps_router[:, 0:KH], chunk[:, g * W1COLS:g * W1COLS + B],
                                     chunk[:, g * W1COLS + B:(g + 1) * W1COLS],
                                     start=(kt == 0), stop=(kt == KT - 1))

            # ---------------- LN1 -> h1T (bf16) + token weights ------------
            h1T = [HTP.tile([P, CP], BF16, tag="hT", name=f"h1T{i}") for i in range(DT)]
            weights_sb = SM.tile([P, CT], F32, tag="weights")
            for t in range(CT):
                x_t = xt(t)
                stats = TMP.tile([P, 6], F32, tag="stats")
                nc.vector.bn_stats(out=stats, in_=x_t)
                mv = TMP.tile([P, 2], F32, tag="mv")
                nc.vector.bn_aggr(out=mv, in_=stats)
                std = TMP.tile([P, 1], F32, tag="std")
                nc.scalar.activation(out=std, in_=mv[:, 1:2], func=AF.Sqrt, bias=eps_t)
                rstd = TMP.tile([P, 1], F32, tag="rstd")
                nc.vector.reciprocal(out=rstd, in_=std)
                h1 = TMP.tile([P, D], F32, tag="h1")
                nc.vector.tensor_scalar(out=h1, in0=x_t, scalar1=mv[:, 0:1], scalar2=rstd,
                                        op0=OP.subtract, op1=OP.mult)
                nc.vector.tensor_mul(h1, h1, ln1g_b)
                h1b = TMP.tile([P, D], BF16, tag="h1b")
                nc.vector.tensor_add(h1b, h1, ln1b_b)
                scr = TMP.tile([P, D], F32, tag="scr", bufs=1)
                nc.vector.scalar_tensor_tensor(out=scr, in0=x_t, scalar=1.0, in1=wpw_b,
                                               op0=OP.mult, op1=OP.mult,
                                               accum_out=weights_sb[:, t:t + 1])
                for dt_ in range(DT):
                    tp = pstb()
                    nc.tensor.transpose(tp[:, 0:P], h1b[:, dt_ * P:(dt_ + 1) * P], ident)
                    nc.vector.tensor_copy(h1T[dt_][:, t * P:(t + 1) * P], tp[:, 0:P])
            nc.vector.tensor_scalar_add(weights_sb, weights_sb, wpb_b[:, 0:1])
            expamask = SM.tile([P, CT], F32, tag="expamask")
            nc.scalar.activation(out=expamask, in_=amask_sb, func=AF.Exp)

            # ---------------- qkT (bf16; heads packed 3-per-tile) ----------
            qh_sb = [QKP.tile([P, CP], BF16, tag=f"qh{j}", name=f"qh{j}") for j in range(NQT)]
            kh_sb = [QKP.tile([P, CP], BF16, tag=f"kh{j}", name=f"kh{j}") for j in range(NQT)]

            def head_slice(tiles, h):
                b = 32 * (h % 3)
                return tiles[h // 3][b:b + HD, :]

            for mt in range(QKM):
                for ncn in range(NQC):
                    ps = pst()
                    for kt in range(DT):
                        nc.tensor.matmul(ps[:, 0:QN], wqk_sb[kt, mt],
                                         h1T[kt][:, ncn * QN:(ncn + 1) * QN],
                                         start=(kt == 0), stop=(kt == DT - 1))
                    for g in range(HPT):
                        h = (mt % DT) * HPT + g
                        dst = head_slice(kh_sb if mt >= DT else qh_sb, h)
                        nc.vector.tensor_scalar(
                            out=dst[:, ncn * QN:(ncn + 1) * QN],
                            in0=ps[g * HD:(g + 1) * HD, 0:QN],
                            scalar1=bqk_sb[mt][g * HD:(g + 1) * HD, :],
                            scalar2=None, op0=OP.add)

            # ---------------- V (bf16, token-major) + ones col -------------
            v_aug = []
            for t in range(CT):
                ps = pst()
                for kt in range(DT):
                    nc.tensor.matmul(ps[:, 0:D], h1T[kt][:, t * P:(t + 1) * P], wv_sb[kt],
                                     start=(kt == 0), stop=(kt == DT - 1))
                va = VAP.tile([P, H, HD + 1], BF16, tag="vaug", name="vaug")
                nc.vector.tensor_add(va[:, :, 0:HD],
                                     ps[:, 0:D].rearrange("p (h d) -> p h d", h=H),
                                     bv_b.rearrange("p (h d) -> p h d", h=H))
                nc.vector.memset(va[:, :, HD:HD + 1], 1.0)
                v_aug.append(va)

            # ---------------- router epilogue -> k -> sel ------------------
            r8 = SM.tile([B, KH], F32, tag="r8")
            nc.vector.tensor_copy(r8, ps_router[:, 0:KH])
            nc.sync.dma_start(out=rs_in.ap(), in_=r8)
            nc.gpsimd.collective_compute(
                "ReduceScatter", OP.add,
                ins=[rs_in.ap()], outs=[rs_out.ap()],
                replica_groups=[list(range(B))],
            )
            klr = SM.tile([1, KH], F32, tag="klr")
            nc.sync.dma_start(out=klr, in_=rs_out.ap())
            nc.vector.tensor_scalar(out=klr, in0=klr, scalar1=1.0 / (SX * SW),
                                    scalar2=None, op0=OP.mult)
            nc.vector.tensor_add(klr, klr, kpb1_sb)
            nc.vector.scalar_tensor_tensor(out=klr, in0=klr, scalar=0.01, in1=klr,
                                            op0=OP.mult, op1=OP.max)   # leaky_relu
            scr2 = SM.tile([1, KH], F32, tag="scr2")
            kl2 = SM.tile([1, 1], F32, tag="kl2")
            nc.vector.scalar_tensor_tensor(out=scr2, in0=klr, scalar=1.0, in1=kpw2_sb,
                                           op0=OP.mult, op1=OP.mult, accum_out=kl2)
            nc.vector.tensor_add(kl2, kl2, kpb2_sb)
            sg = SM.tile([1, 1], F32, tag="sg")
            nc.scalar.activation(out=sg, in_=kl2, func=AF.Exp, scale=-1.0)
            nc.vector.tensor_scalar_add(sg, sg, 1.0)
            nc.vector.reciprocal(sg, sg)
            kv = SM.tile([1, 1], F32, tag="kv")
            nc.vector.tensor_scalar(out=kv, in0=sg, scalar1=float(S), scalar2=1.0,
                                    op0=OP.mult, op1=OP.max)
            nc.vector.tensor_scalar_min(kv, kv, float(S))
            # k = floor(clip(sig*S,1,S)) == count of j in [1,S] with j <= v
            psv = pst()
            nc.tensor.matmul(psv[0:4, 0:1], ones_row[:, 0:4], kv, start=True, stop=True)
            vb4 = SM.tile([4, 1], F32, tag="vb4")
            nc.vector.tensor_copy(vb4, psv[0:4, 0:1])
            kcmp = SM.tile([4, S // 4], F32, tag="kcmp")
            nc.vector.tensor_single_scalar(out=kcmp, in_=iota_sb, scalar=vb4, op=OP.is_le)
            cnt4 = SM.tile([4, 1], F32, tag="cnt4")
            nc.vector.tensor_reduce(out=cnt4, in_=kcmp, axis=mybir.AxisListType.X, op=OP.add)
            psc = pst()
            nc.tensor.matmul(psc[0:1, 0:1], ones4, cnt4, start=True, stop=True)
            nc.vector.tensor_copy(kv, psc[0:1, 0:1])
            psk = pst()
            nc.tensor.matmul(psk[:, 0:1], ones_row, kv, start=True, stop=True)
            kb = SM.tile([P, 1], F32, tag="kb")
            nc.vector.tensor_copy(kb, psk[:, 0:1])
            sel01 = SM.tile([P, CT], F32, tag="sel01")
            nc.vector.tensor_single_scalar(out=sel01, in_=weights_sb, scalar=kb[:, 0:1], op=OP.is_gt)
            m01 = SM.tile([P, CT], F32, tag="m01")
            nc.vector.tensor_mul(m01, sel01, expamask)
            wsel = SM.tile([P, CT], F32, tag="wsel")
            nc.vector.tensor_mul(wsel, weights_sb, sel01)
            for t in range(CT):
                nc.vector.tensor_scalar_mul(v_aug[t], v_aug[t], m01[:, t:t + 1])

            # ---------------- attention ----------------
            scale = 1.0 / float(np.sqrt(HD))
            attnT = [ATP.tile([P, CP], BF16, tag=f"attnT{ht}", name=f"attnT{ht}") for ht in range(DT)]
            den_sb = SM.tile([H, CP], F32, tag="den")
            for h in range(H):
                ht, hr = divmod(h, HPT)
                qT = head_slice(qh_sb, h)
                kT = head_slice(kh_sb, h)
                for sqc in range(NSQC):
                    pv = pst(HD + 1)
                    for skt in range(CT):
                        ps_s = pst()
                        for (off, w_) in PIECES:
                            nc.tensor.matmul(ps_s[:, off:off + w_],
                                             kT[:, skt * P:(skt + 1) * P],
                                             qT[:, sqc * CH + off:sqc * CH + off + w_],
                                             start=True, stop=True)
                        et = EXP.tile([P, CH], BF16, tag="expT", name="expT")
                        nc.scalar.activation(out=et, in_=ps_s[:, 0:CH], func=AF.Exp, scale=scale)
                        for (off, w_) in PIECES:
                            nc.tensor.matmul(pv[:, off:off + w_],
                                             v_aug[skt][:, h, :],
                                             et[:, off:off + w_],
                                             start=(skt == 0), stop=(skt == CT - 1))
                    nc.vector.tensor_copy(attnT[ht][hr * HD:(hr + 1) * HD, sqc * CH:(sqc + 1) * CH],
                                          pv[0:HD, 0:CH])
                    dstg = TMP.tile([1, CH], F32, tag="dstg", bufs=2)
                    nc.vector.tensor_copy(dstg, pv[HD:HD + 1, 0:CH])
                    nc.sync.dma_start(out=den_sb[h:h + 1, sqc * CH:(sqc + 1) * CH], in_=dstg)
            nc.vector.tensor_scalar_add(den_sb, den_sb, NEG_EPS)
            den_bf = SM.tile([H, CP], BF16, tag="denbf")
            with nc.allow_low_precision(reason="bf16 softmax denominators within 2e-2 gate"):
                nc.vector.reciprocal(den_bf, den_sb)
            for ht in range(DT):
                for sqc in range(NSQC):
                    psb_ = pst()
                    for (off, w_) in PIECES:
                        nc.tensor.matmul(psb_[:, off:off + w_], E_sb[ht],
                                         den_bf[:, sqc * CH + off:sqc * CH + off + w_],
                                         start=True, stop=True)
                    sl = attnT[ht][:, sqc * CH:(sqc + 1) * CH]
                    nc.vector.tensor_mul(sl, sl, psb_[:, 0:CH])

            # ---------------- a = x + attn@wo + bo ; LN2 -> h2T (bf16) -----
            h2T = [HTP.tile([P, CP], BF16, tag="hT", name=f"h2T{i}") for i in range(DT)]
            a_sb = []
            for t in range(CT):
                ps = pst()
                for kt in range(DT):
                    nc.tensor.matmul(ps[:, 0:D], attnT[kt][:, t * P:(t + 1) * P], wo_sb[kt],
                                     start=(kt == 0), stop=(kt == DT - 1))
                a_t = APL.tile([P, D], F32, tag="a", name="a")
                nc.vector.scalar_tensor_tensor(out=a_t, in0=ps[:, 0:D], scalar=1.0, in1=xt(t),
                                               op0=OP.mult, op1=OP.add)
                nc.vector.tensor_add(a_t, a_t, bo_b)
                a_sb.append(a_t)
                stats = TMP.tile([P, 6], F32, tag="stats")
                nc.vector.bn_stats(out=stats, in_=a_t)
                mv = TMP.tile([P, 2], F32, tag="mv")
                nc.vector.bn_aggr(out=mv, in_=stats)
                std = TMP.tile([P, 1], F32, tag="std")
                nc.scalar.activation(out=std, in_=mv[:, 1:2], func=AF.Sqrt, bias=eps_t)
                rstd = TMP.tile([P, 1], F32, tag="rstd")
                nc.vector.reciprocal(out=rstd, in_=std)
                h2 = TMP.tile([P, D], F32, tag="h1")
                nc.vector.tensor_scalar(out=h2, in0=a_t, scalar1=mv[:, 0:1], scalar2=rstd,
                                        op0=OP.subtract, op1=OP.mult)
                nc.vector.tensor_mul(h2, h2, ln2g_b)
                h2b = TMP.tile([P, D], BF16, tag="h1b")
                nc.vector.tensor_add(h2b, h2, ln2b_b)
                for dt_ in range(DT):
                    tp = pstb()
                    nc.tensor.transpose(tp[:, 0:P], h2b[:, dt_ * P:(dt_ + 1) * P], ident)
                    nc.vector.tensor_copy(h2T[dt_][:, t * P:(t + 1) * P], tp[:, 0:P])

            # ---------------- MLP (bf16, native gelu) ----------------
            out_sb = XSB.tile([P, CT * D], F32, tag="outsb", name="outsb")
            for t in range(CT):
                ps2 = pst()
                for mt in range(MT):
                    ps = pst()
                    for kt in range(DT):
                        nc.tensor.matmul(ps[:, 0:P], wm1_sb[kt, mt],
                                         h2T[kt][:, t * P:(t + 1) * P],
                                         start=(kt == 0), stop=(kt == DT - 1))
                    pre = M1P.tile([P, P], F32, tag="m1pre", name="m1pre")
                    nc.vector.tensor_scalar(out=pre, in0=ps[:, 0:P], scalar1=bm1_sb[mt],
                                            scalar2=None, op0=OP.add)
                    s = M1P.tile([P, P], F32, tag="m1s", name="m1s")
                    nc.gpsimd.tensor_mul(s, pre, pre)
                    nc.gpsimd.tensor_scalar(out=s, in0=s, scalar1=0.044715, scalar2=1.0,
                                            op0=OP.mult, op1=OP.add)
                    nc.gpsimd.tensor_mul(s, s, pre)
                    nc.scalar.activation(out=s, in_=s, func=AF.Tanh, scale=GELU_C)
                    g = M1P.tile([P, P], BF16, tag="m1g", name="m1g")
                    nc.vector.scalar_tensor_tensor(out=g, in0=s, scalar=1.0, in1=pre,
                                                   op0=OP.add, op1=OP.mult)
                    nc.tensor.matmul(ps2[:, 0:D], g, wm2_sb[mt],
                                     start=(mt == 0), stop=(mt == MT - 1))
                f1 = TMP.tile([P, D], F32, tag="f1", bufs=2)
                nc.vector.scalar_tensor_tensor(out=f1, in0=ps2[:, 0:D],
                                               scalar=1.0, in1=a_sb[t],
                                               op0=OP.mult, op1=OP.add)
                nc.vector.tensor_add(f1, f1, bm2_b)
                nc.vector.scalar_tensor_tensor(out=out_sb[:, t * D:(t + 1) * D],
                                               in0=f1, scalar=wsel[:, t:t + 1],
                                               in1=xt(t), op0=OP.mult, op1=OP.add)
            nc.sync.dma_start(out=out_d.ap(), in_=out_sb)

    nc.compile()
    return nc


def _weights_host(inputs):
    x = np.asarray(inputs["x"], np.float32)
    wp_w = np.asarray(inputs["wp_w"], np.float32)
    wp_b = np.asarray(inputs["wp_b"], np.float32)
    return (x @ wp_w + wp_b)[..., 0]          # [B, S]


def choose_cp(inputs):
    w = _weights_host(inputs)
    cnt = (w > 1.0 - W_MARGIN).sum(axis=1).max()
    CP = int(min(FULL_CFG["S"], max(256, int(np.ceil(cnt / 128.0)) * 128)))
    return CP


def marshal_inputs(cfg, inputs, CP):
    S, D, H, MLP_, KH, B = (cfg[k] for k in ("S", "D", "H", "MLP", "KH", "B"))
    P = 128
    CT = CP // P
    KSLICE = S * D // B
    KT = KSLICE // P
    W1COLS = B + KH

    f = lambda k: np.asarray(inputs[k], dtype=np.float32)
    bf = lambda a: np.ascontiguousarray(a).astype(ml_dtypes.bfloat16)
    x = f("x")
    amask = f("attention_mask")
    kp_w1 = f("kp_w1")
    x_flat = x.reshape(B, S * D)

    w = _weights_host(inputs)
    perms = [np.argsort(-w[i], kind="stable") for i in range(B)]

    DT = D // P
    QKM = 2 * D // P
    MT = MLP_ // P
    wqk = f("wqkv")[:, :2 * D]
    wv = f("wqkv")[:, 2 * D:]
    wo_ = f("wo")
    wm1 = f("wm1")
    wm2h = 0.5 * f("wm2")
    tiles = []
    for kt in range(DT):
        for mt in range(QKM):
            tiles.append(wqk[kt * P:(kt + 1) * P, mt * P:(mt + 1) * P])
    for kt in range(DT):
        tiles.append(wv[kt * P:(kt + 1) * P, :])
    for kt in range(DT):
        tiles.append(wo_[kt * P:(kt + 1) * P, :])
    for kt in range(DT):
        for mt in range(MT):
            tiles.append(wm1[kt * P:(kt + 1) * P, mt * P:(mt + 1) * P])
    for mt in range(MT):
        tiles.append(wm2h[mt * P:(mt + 1) * P, :])
    wpack = bf(np.concatenate(tiles, axis=1))

    rowpack = np.concatenate([
        f("ln1_g").reshape(-1), f("ln1_b").reshape(-1),
        f("ln2_g").reshape(-1), f("ln2_b").reshape(-1),
        f("wp_w").reshape(-1), f("bqkv")[2 * D:].reshape(-1),
        f("bo").reshape(-1), f("bm2").reshape(-1),
        np.full(D, float(f("wp_b").reshape(-1)[0]), np.float32),
        f("kp_b1").reshape(-1), f("kp_w2").reshape(-1),
        f("kp_b2").reshape(-1), np.zeros(1, np.float32),
    ]).reshape(1, -1).astype(np.float32)

    colpack = np.concatenate(
        [f("bqkv")[:2 * D].reshape(QKM, P).T] +
        [f("bm1").reshape(MT, P).T], axis=1).astype(np.float32)

    shared = dict(wpack=wpack, rowpack=rowpack, colpack=colpack)
    HD = D // H
    HPT = P // HD
    Es = []
    for ht in range(D // P):
        E = np.zeros((H, P), np.float32)
        for hr in range(HPT):
            h = ht * HPT + hr
            if h < H:
                E[h, hr * HD:(hr + 1) * HD] = 1.0
        Es.append(E)
    shared["Emats"] = np.concatenate(Es, axis=1).astype(ml_dtypes.bfloat16)
    shared["iota_row"] = np.arange(1, S + 1, dtype=np.float32).reshape(4, S // 4)

    fp8 = ml_dtypes.float8_e4m3
    in_maps = []
    for i in range(B):
        sl = slice(i * KSLICE, (i + 1) * KSLICE)
        xr = np.ascontiguousarray(x_flat[:, sl].T) * SX
        w1s = kp_w1[sl] * SW
        w1x = np.concatenate(
            [xr.reshape(KT, P, B).astype(fp8), w1s.reshape(KT, P, KH).astype(fp8)],
            axis=2)
        w1x = np.ascontiguousarray(w1x.transpose(1, 0, 2).reshape(P, KT * W1COLS))
        pre = perms[i][:CP]
        x_sel = x[i][pre]                                   # [CP, D]
        m = dict(shared)
        m["x_sel_pm"] = np.ascontiguousarray(
            x_sel.reshape(CT, P, D).transpose(1, 0, 2).reshape(P, CT * D))
        m["w1x"] = w1x
        m["amask_sel"] = np.ascontiguousarray(
            amask[i, 0, 0][pre].reshape(CT, P).T)
        in_maps.append(m)
    return in_maps, perms


_NC_CACHE = {}


def _get_nc(CP, repeat=1):
    key = (CP, repeat)
    if key not in _NC_CACHE:
        _NC_CACHE[key] = build_bass(FULL_CFG, CP, repeat=repeat)
    return _NC_CACHE[key]


def prepare(inputs, repeat=1):
    cfg = FULL_CFG
    CP = choose_cp(inputs)
    nc = _get_nc(CP, repeat)
    in_maps, perms = marshal_inputs(cfg, inputs, CP)
    return nc, in_maps, perms, CP


def run(inputs, trace=False, repeat=1, **kw):
    from concourse.bass_utils import run_bass_kernel_spmd

    cfg = FULL_CFG
    S, D = cfg["S"], cfg["D"]
    P = 128
    nc, in_maps, perms, CP = prepare(inputs, repeat)
    CT = CP // P
    res = run_bass_kernel_spmd(nc, in_maps, list(range(cfg["B"])), trace=trace, **kw)
    x = np.asarray(inputs["x"], np.float32)
    out = x.copy()
    for i in range(cfg["B"]):
        o = res.results[i]["out"].reshape(P, CT, D).transpose(1, 0, 2).reshape(CP, D)
        out[i][perms[i][:CP]] = o
    return out.astype(np.float32), res


def kernel(**inputs):
    return run(inputs)[0]
